# revision 1
# baseline (speedup 1.0000x reference)
"""Trainium2 Bass kernel for Conv2dWeightModulate (no style).

The reference computes an equalized-lr + demodulated 3x3 conv:
    w = weight * C_EQ;  w *= rsqrt(sum(w^2, (I,K,K)) + eps);  out = conv2d(x, w, pad=1)

The tiny weight normalization runs on host (numpy); the conv runs on 8
NeuronCores, data-parallel over the batch (2 images per core).

Host-side data layout: x is cast to bf16 and split by row parity into
xP[b, 128, h2, w+2] where partition k = 64*p + c holds x[b, c, 2*h2+p, w]
(p = row parity), rows pre-padded to 258 columns with zero borders.  Both
parity halves share the same h2 indexing, so each block's SBUF x tile
loads with single full-128-partition DMAs (all 16 SDMA engines engaged).

Device kernel layout (per core):
  x is stored in SBUF parity-interleaved: partitions 0-63 hold the 64
  channels of even image rows, partitions 64-127 the odd rows.  Chunk
  column s of a block with row base R holds h2 row R/2 - 1 + s of both
  halves, i.e.
      half A (parts 0:64):   x row R + 2(s-1)
      half B (parts 64:128): x row R + 2s - 1
  so chunk s aligns x rows (2j, 2j+1) vertically.  Per 8 output rows
  (free dim 512 = 2 rows x 256), the 3x3 conv is:
      - mains: even rows (taps kh=1,2) and odd rows (taps kh=0,1) read the
        SAME rhs chunks, so they fuse into one full-array M=128 matmul
        (E outputs in one 64-col group, O in the other) x3 kw x2 row-pairs.
        One 128-col LDWEIGHTS per matmul keeps the weight path ahead of
        the 213ns N=512 streaming slots.
      - leftovers: the third tap of each parity is K=64; the four of them
        (E1,O1,E2,O2) land on the four disjoint PE quadrants per kw and
        run concurrently.
  Bank 1 holds (E1|O1) rows, bank 2 (O2|E2) so the quadrants stay
  disjoint; each bank evacuates with a single 128-partition copy (ACT for
  bank 1, DVE for bank 2 - never the same bank).  Accumulation is fp32 in
  PSUM; outputs stage through SBUF as bf16 and DMA out as bf16 into a
  [b, c, p, uh, hg, uu, up, w] layout (h2 = 16*hg + 4*uu + 2*uh + up)
  chosen so every channel's store is one contiguous 4 KiB span; the host
  re-interleaves and upcasts to fp32.  The final group streams per-uu so
  the kernel does not end on a large store drain.
"""

import numpy as np

IN_F = 64
OUT_F = 64
KS = 3
EPS = 1e-05
C_EQ = 1.0 / np.sqrt(IN_F * KS * KS)

B_FULL = 16
H_FULL = 256
W = 256
N_CORES = 8
CW = W + 2  # padded row width


def build_nc(bpc, h, block=64, out_bf16=True):
    """Build the per-core Bass program: bpc images of [64, h, 256] each."""
    from concourse import bacc
    import concourse.mybir as mybir
    from concourse.tile import TileContext

    assert h % block == 0 and block % 32 == 0
    nblk = h // block
    ngrp = block // 32  # 32-row output staging groups per block
    sch = block // 2 + 2  # chunk columns per x tile
    hgs = h // 32  # 32-row output groups per image
    f32 = mybir.dt.float32
    bf16 = mybir.dt.bfloat16

    nc = bacc.Bacc("TRN2", target_bir_lowering=False, debug=False)
    # partition k = 64*p + c  (p = row parity, c = channel)
    x = nc.dram_tensor("x", [bpc, 2 * IN_F, h // 2, CW], bf16, kind="ExternalInput")
    wp = nc.dram_tensor("wpack", [128, 9, 128], bf16, kind="ExternalInput")
    odt = bf16 if out_bf16 else f32
    # out h2 index decomposed as 16*hg + 4*uu + 2*uh + up; uh is the PSUM
    # bank index, so per (b, c, p, uh, hg) the (uu, up, w) span is one
    # contiguous run in DRAM.
    out = nc.dram_tensor(
        "out", [bpc, OUT_F, 2, 2, hgs, 4, 2, W], odt, kind="ExternalOutput"
    )

    with TileContext(nc) as tc:
        with (
            tc.tile_pool(name="xp", bufs=4) as xpool,
            tc.tile_pool(name="wpool", bufs=1) as wpool,
            tc.tile_pool(name="st", bufs=3) as spool,
            tc.tile_pool(name="ps", bufs=4, space="PSUM") as ppool,
        ):
            wt = wpool.tile([128, 9, 128], bf16)
            nc.sync.dma_start(out=wt[:], in_=wp.ap())
            for b in range(bpc):
                for blk in range(nblk):
                    R = blk * block
                    h0 = R // 2
                    xt = xpool.tile([128, sch, CW], bf16, tag="xt")
                    # chunk s of both halves <- h2 row (h0-1)+s; boundary
                    # chunks that fall outside the image are either zeroed
                    # (read as conv padding) or skipped (never read).
                    if blk == 0:
                        c_lo, c_hi = 1, sch  # A chunk 0 unused; B chunk 0 zero
                        nc.gpsimd.memset(xt[64:128, 0, :], 0.0)
                    elif blk == nblk - 1:
                        c_lo, c_hi = 0, sch - 1  # B chunk sch-1 unused
                        nc.gpsimd.memset(xt[0:64, sch - 1, :], 0.0)
                    else:
                        c_lo, c_hi = 0, sch
                    r_lo = h0 - 1 + c_lo
                    ntot = c_hi - c_lo
                    # the very first tile streams in small pieces so compute
                    # starts as soon as the first chunks land
                    nsplit = 6 if (b == 0 and blk == 0) else 2
                    bounds = [c_lo + (ntot * i) // nsplit for i in range(nsplit + 1)]
                    for s_lo, s_hi in zip(bounds[:-1], bounds[1:]):
                        nc.sync.dma_start(
                            out=xt[:, s_lo:s_hi, :],
                            in_=x.ap()[b, :, s_lo - c_lo + r_lo : s_hi - c_lo + r_lo, :],
                        )
                    for g in range(ngrp):
                        hg = (h0 + 16 * g) // 16
                        last_g = b == bpc - 1 and blk == nblk - 1 and g == ngrp - 1
                        st1 = spool.tile([128, 4, 2, W], odt, tag="st1")
                        st2 = spool.tile([128, 4, 2, W], odt, tag="st2")
                        # bank1(uu) = (E1 | O1) rows, bank2(uu) = (O2 | E2)
                        ps = [
                            (
                                ppool.tile([128, 2, W], f32, tag="ps1", name="ps1"),
                                ppool.tile([128, 2, W], f32, tag="ps2", name="ps2"),
                            )
                            for _ in range(4)
                        ]
                        # phase 1 - fused mains for all four uu: E rows
                        # (kh=1,2) + O rows (kh=0,1) share the rhs stream ->
                        # one M=128 full-array matmul per (uu, pair, kw).
                        # Keeping all 24 in one burst avoids the ~95ns PE
                        # reconfiguration at every mains<->leftover switch.
                        for uu in range(4):
                            s0 = 16 * g + 4 * uu + 1  # A-chunk of x row r0
                            ps1, ps2 = ps[uu]
                            for kw in range(3):
                                st_ = kw == 0
                                nc.tensor.matmul(
                                    ps1[0:128], wt[:, kw, :],
                                    xt[:, s0 : s0 + 2, kw : kw + W],
                                    start=st_, stop=False,
                                )
                                nc.tensor.matmul(
                                    ps2[0:128], wt[:, 3 + kw, :],
                                    xt[:, s0 + 2 : s0 + 4, kw : kw + W],
                                    start=st_, stop=False,
                                )
                        # phase 2 - K=64 leftovers: E tap kh=0 from half B,
                        # O tap kh=2 from half A; per kw the four land on
                        # disjoint quadrants (64,0) (0,64) (64,64) (0,0)
                        for uu in range(4):
                            s0 = 16 * g + 4 * uu + 1
                            ps1, ps2 = ps[uu]
                            for kw in range(3):
                                sp_ = kw == 2
                                nc.tensor.matmul(
                                    ps1[0:64], wt[64:128, 6 + kw, 0:64],
                                    xt[64:128, s0 - 1 : s0 + 1, kw : kw + W],
                                    start=False, stop=sp_,
                                )
                                nc.tensor.matmul(
                                    ps1[64:128], wt[0:64, 6 + kw, 0:64],
                                    xt[0:64, s0 + 1 : s0 + 3, kw : kw + W],
                                    start=False, stop=sp_,
                                )
                                nc.tensor.matmul(
                                    ps2[64:128], wt[64:128, 6 + kw, 64:128],
                                    xt[64:128, s0 + 1 : s0 + 3, kw : kw + W],
                                    start=False, stop=sp_,
                                )
                                nc.tensor.matmul(
                                    ps2[0:64], wt[0:64, 6 + kw, 64:128],
                                    xt[0:64, s0 + 3 : s0 + 5, kw : kw + W],
                                    start=False, stop=sp_,
                                )
                            nc.scalar.copy(st1[:, uu], ps1[:])
                            nc.vector.tensor_copy(out=st2[:, uu], in_=ps2[:])
                            if last_g:
                                # stream the final group per-uu, split across
                                # both HWDGE queues (sync + scalar): each
                                # dma_start occupies its engine ~0.6us, so one
                                # queue alone would serialize ~10us of issue
                                nc.sync.dma_start(
                                    out=out.ap()[b, :, 0, 0, hg, uu], in_=st1[0:64, uu]
                                )
                                nc.scalar.dma_start(
                                    out=out.ap()[b, :, 1, 0, hg, uu], in_=st1[64:128, uu]
                                )
                                nc.sync.dma_start(
                                    out=out.ap()[b, :, 1, 1, hg, uu], in_=st2[0:64, uu]
                                )
                                nc.scalar.dma_start(
                                    out=out.ap()[b, :, 0, 1, hg, uu], in_=st2[64:128, uu]
                                )
                        if not last_g:
                            # group output DMAs all on gpsimd: sync stays free
                            # to prefetch the next block's x tiles
                            nc.gpsimd.dma_start(
                                out=out.ap()[b, :, 0, 0, hg], in_=st1[0:64]
                            )
                            nc.gpsimd.dma_start(
                                out=out.ap()[b, :, 1, 0, hg], in_=st1[64:128]
                            )
                            nc.gpsimd.dma_start(
                                out=out.ap()[b, :, 1, 1, hg], in_=st2[0:64]
                            )
                            nc.gpsimd.dma_start(
                                out=out.ap()[b, :, 0, 1, hg], in_=st2[64:128]
                            )
    nc.compile()
    return nc


def normalize_weight(weight):
    """Host-side equalized-lr + demodulation of the [O,I,3,3] weight."""
    w = np.asarray(weight, dtype=np.float32) * np.float32(C_EQ)
    sigma_inv = 1.0 / np.sqrt(
        np.sum((w * w).astype(np.float32), axis=(1, 2, 3), keepdims=True) + EPS
    )
    return (w * sigma_inv.astype(np.float32)).astype(np.float32)


def pack_weights(w_norm):
    """Pack normalized [O,I,kh,kw] weights into the [128, 9, 128] SBUF image.

    Slot kw (0..2) is the fused main weight for row-pair 1: cols 0:64 are
    the even-row mains (rows 0:64 <- kh=1, rows 64:128 <- kh=2), cols
    64:128 the odd-row mains (kh=0 / kh=1).  Slot 3+kw is the same for
    row-pair 2 with the column halves swapped (O | E).  Slot 6+kw holds
    the K=64 leftovers: cols 0:64 rows 0:64 <- kh=2 (O tap), rows 64:128
    <- kh=0 (E tap); cols 64:128 duplicate them for the second row-pair's
    quadrants.  Each [64, 64] sub-block is w[:, :, kh, kw].T (contraction
    dim first).
    """
    wt = np.transpose(w_norm, (2, 3, 1, 0))  # [kh, kw, in, out]
    wpack = np.zeros((128, 9, 128), dtype=np.float32)
    for kw in range(3):
        # fused mains, row-pair 1: [E | O]
        wpack[0:64, kw, 0:64] = wt[1, kw]
        wpack[64:128, kw, 0:64] = wt[2, kw]
        wpack[0:64, kw, 64:128] = wt[0, kw]
        wpack[64:128, kw, 64:128] = wt[1, kw]
        # fused mains, row-pair 2: [O | E]
        wpack[0:64, 3 + kw, 0:64] = wt[0, kw]
        wpack[64:128, 3 + kw, 0:64] = wt[1, kw]
        wpack[0:64, 3 + kw, 64:128] = wt[1, kw]
        wpack[64:128, 3 + kw, 64:128] = wt[2, kw]
        # leftovers (both col-halves identical)
        for half in (0, 64):
            wpack[0:64, 6 + kw, half : half + 64] = wt[2, kw]
            wpack[64:128, 6 + kw, half : half + 64] = wt[0, kw]
    return wpack


_NC_CACHE = {}


def _get_nc(bpc, h, block=64, out_bf16=True):
    key = (bpc, h, block, out_bf16)
    if key not in _NC_CACHE:
        _NC_CACHE[key] = build_nc(bpc, h, block, out_bf16)
    return _NC_CACHE[key]


def split_parity(x_f32):
    """[b, c, h, w] f32 -> bf16 [b, 2*c, h//2, w+2]: row parity split plus
    zero border columns; partition k = 64*p + c (p=0 even rows, p=1 odd)."""
    import ml_dtypes

    b, c, h, w = x_f32.shape
    xb = x_f32.astype(ml_dtypes.bfloat16)
    xP = np.zeros((b, 2, c, h // 2, w + 2), dtype=ml_dtypes.bfloat16)
    xP[:, 0, :, :, 1:-1] = xb[:, :, 0::2]
    xP[:, 1, :, :, 1:-1] = xb[:, :, 1::2]
    return xP.reshape(b, 2 * c, h // 2, w + 2)


def merge_parity(outP):
    """[b, c, 2, 2, hgs, 4, 2, w] (any float dtype) -> fp32 [b, c, h, w]
    with h = 2*(16*hg + 4*uu + 2*uh + up) + p."""
    b, c, _, _, hgs, _, _, w = outP.shape
    # axes: b c p uh hg uu up w -> b c hg uu uh up p w
    o = np.transpose(outP, (0, 1, 4, 5, 3, 6, 2, 7))
    return np.ascontiguousarray(o, dtype=np.float32).reshape(b, c, 32 * hgs, w)


def kernel(x, weight):
    import ml_dtypes
    from concourse import bass_utils

    x = np.asarray(x, dtype=np.float32)
    weight = np.asarray(weight, dtype=np.float32)
    assert x.shape == (B_FULL, IN_F, H_FULL, W), x.shape

    xP = split_parity(x)
    wpack = pack_weights(normalize_weight(weight)).astype(ml_dtypes.bfloat16)
    bpc = B_FULL // N_CORES
    nc = _get_nc(bpc, H_FULL)
    in_maps = [
        {"x": xP[i * bpc : (i + 1) * bpc], "wpack": wpack} for i in range(N_CORES)
    ]
    res = bass_utils.run_bass_kernel_spmd(nc, in_maps, core_ids=list(range(N_CORES)))
    return np.concatenate([merge_parity(r["out"]) for r in res.results], axis=0)



# revision 7
# speedup vs baseline: 1.2063x; 1.2063x over previous
"""Trainium2 Bass kernel for Conv2dWeightModulate (no style).

The reference computes an equalized-lr + demodulated 3x3 conv:
    w = weight * C_EQ;  w *= rsqrt(sum(w^2, (I,K,K)) + eps);  out = conv2d(x, w, pad=1)

The tiny weight normalization runs on host (numpy); the conv runs on 8
NeuronCores, data-parallel over the batch (2 images per core).

Host-side data layout: x is cast to bf16 and split by row parity into
xP[b, 128, h2, w+2] where partition k = 64*p + c holds x[b, c, 2*h2+p, w]
(p = row parity), rows pre-padded to 258 columns with zero borders.  Both
parity halves share the same h2 indexing, so each block's SBUF x tile
loads with single full-128-partition DMAs (all 16 SDMA engines engaged).

Device kernel layout (per core):
  x is stored in SBUF parity-interleaved: partitions 0-63 hold the 64
  channels of even image rows, partitions 64-127 the odd rows.  Chunk
  column s of a block with row base R holds h2 row R/2 - 1 + s of both
  halves, so chunk s aligns x rows (2j, 2j+1) vertically.  Per 8 output
  rows (free dim 512 = 2 rows x 256), the 3x3 conv is:
      - mains: even rows (taps kh=1,2) and odd rows (taps kh=0,1) read the
        SAME rhs chunks, so they fuse into one full-array M=128 matmul
        (E outputs in one 64-col group, O in the other) x3 kw x2 row-pairs.
      - leftovers: the third tap of each parity is K=64; the four of them
        (E1,O1,E2,O2) land on the four disjoint PE quadrants per kw and
        run concurrently.
  Switching the PE between the full-array mains and the quadrant
  leftovers costs a full-array drain (~190ns); groups alternate
  (mains,leftovers) / (leftovers,mains) order so consecutive groups
  share a mode at the boundary and only ONE switch per group remains.
  In reversed groups the PSUM accumulation starts on the first quadrant
  matmul per bank (its whole-bank has_written clear lands before the
  4ns-later sibling quadrant's first write drains) and stops on the
  kw=2 mains.
  Bank 1 holds (E1|O1) rows, bank 2 (O2|E2) so the quadrants stay
  disjoint; each bank evacuates with a single 128-partition copy (ACT for
  bank 1, DVE for bank 2 - never the same bank).  Accumulation is fp32 in
  PSUM; outputs stage through SBUF as bf16 and DMA out as bf16 with a
  single 128-partition store per staging tile into
  out[b, uh, k, hg, uu, up, w] where partition k = 64*ph + c, row parity
  p = ph ^ uh and h2 = 16*hg + 4*uu + 2*uh + up; every partition's store
  is one contiguous 4 KiB span.  The final group streams per-uu split
  over the sync+scalar HWDGE queues so the kernel does not end on a
  large store drain.
  Startup: the mains weights + first x chunks stream on sync while the
  leftover weights + later chunks go via scalar, and three throwaway
  matmuls on a zeroed scratch tile start the PE HAM warmup window while
  the first x chunks are still in flight.
"""

import numpy as np

IN_F = 64
OUT_F = 64
KS = 3
EPS = 1e-05
C_EQ = 1.0 / np.sqrt(IN_F * KS * KS)

B_FULL = 16
H_FULL = 256
W = 256
N_CORES = 8
CW = W + 2  # padded row width


def build_nc(bpc, h, block=64, out_bf16=True):
    """Build the per-core Bass program: bpc images of [64, h, 256] each."""
    from concourse import bacc
    import concourse.mybir as mybir
    from concourse.tile import TileContext

    assert h % block == 0 and block % 32 == 0
    nblk = h // block
    ngrp = block // 32  # 32-row output staging groups per block
    sch = block // 2 + 2  # chunk columns per x tile
    hgs = h // 32  # 32-row output groups per image
    f32 = mybir.dt.float32
    bf16 = mybir.dt.bfloat16

    nc = bacc.Bacc("TRN2", target_bir_lowering=False, debug=False)
    # partition k = 64*p + c  (p = row parity, c = channel)
    x = nc.dram_tensor("x", [bpc, 2 * IN_F, h // 2, CW], bf16, kind="ExternalInput")
    wp = nc.dram_tensor("wpack", [128, 9, 128], bf16, kind="ExternalInput")
    odt = bf16 if out_bf16 else f32
    # out[b, uh, k, hg, uu, up, w]: partition k = 64*ph + c, parity
    # p = ph ^ uh, h2 = 16*hg + 4*uu + 2*uh + up, image row = 2*h2 + p.
    out = nc.dram_tensor(
        "out", [bpc, 2, 128, hgs, 4, 2, W], odt, kind="ExternalOutput"
    )

    with TileContext(nc) as tc:
        with (
            tc.tile_pool(name="xp", bufs=4) as xpool,
            tc.tile_pool(name="wpool", bufs=1) as wpool,
            tc.tile_pool(name="wm", bufs=1) as wmpool,
            tc.tile_pool(name="st", bufs=3) as spool,
            tc.tile_pool(name="ps", bufs=4, space="PSUM") as ppool,
        ):
            wt = wpool.tile([128, 9, 128], bf16)
            # mains weights (slots 0-5) via sync; leftovers via scalar (the
            # scalar queue opens later, behind the preamble ACT_TABLE_LOAD)
            nc.sync.dma_start(out=wt[:, 0:6, :], in_=wp.ap()[:, 0:6, :])
            nc.scalar.dma_start(out=wt[:, 6:9, :], in_=wp.ap()[:, 6:9, :])

            # PE warmup: start the HAM activity window while the first x
            # chunks are still in flight (results are discarded).  The
            # warmup matmuls alternate PSUM banks: a start=True bank clear
            # must never fire while the previous matmul is still draining
            # into the same bank.
            wmt = wmpool.tile([128, 2, W], bf16)
            nc.gpsimd.memset(wmt[:], 0.0)
            pw1 = ppool.tile([128, 2, W], f32, tag="ps1", name="psw1")
            pw2 = ppool.tile([128, 2, W], f32, tag="ps2", name="psw2")
            for i in range(3):
                nc.tensor.matmul(
                    (pw1 if i % 2 == 0 else pw2)[:], wmt[:, 0, 0:128], wmt[:],
                    start=True, stop=True,
                )

            gidx = 0
            for b in range(bpc):
                for blk in range(nblk):
                    R = blk * block
                    h0 = R // 2
                    xt = xpool.tile([128, sch, CW], bf16, tag="xt")
                    # chunk s of both halves <- h2 row (h0-1)+s; boundary
                    # chunks that fall outside the image are either zeroed
                    # (read as conv padding) or skipped (never read).
                    if blk == 0:
                        c_lo, c_hi = 1, sch  # A chunk 0 unused; B chunk 0 zero
                        nc.gpsimd.memset(xt[64:128, 0, :], 0.0)
                    elif blk == nblk - 1:
                        c_lo, c_hi = 0, sch - 1  # B chunk sch-1 unused
                        nc.gpsimd.memset(xt[0:64, sch - 1, :], 0.0)
                    else:
                        c_lo, c_hi = 0, sch
                    r_lo = h0 - 1 + c_lo
                    # the very first tile streams in small pieces so compute
                    # starts as soon as the first chunks land.  ALL x pieces
                    # stay on the sync HWDGE ring: one ring completes FIFO,
                    # so the early chunks finish first; a second ring's
                    # packets would round-robin with these on the 16 SDMA
                    # engines and starve the critical first pieces.
                    if b == 0 and blk == 0:
                        pieces = [(1, 3, 0), (3, 6, 0), (6, 10, 0), (10, 16, 0),
                                  (16, 23, 0), (23, 34, 0)]
                    else:
                        mid = (c_lo + c_hi) // 2
                        pieces = [(c_lo, mid, 0), (mid, c_hi, 0)]
                    for s_lo, s_hi, eng in pieces:
                        e = nc.scalar if eng else nc.sync
                        e.dma_start(
                            out=xt[:, s_lo:s_hi, :],
                            in_=x.ap()[b, :, s_lo - c_lo + r_lo : s_hi - c_lo + r_lo, :],
                        )
                    for g in range(ngrp):
                        hg = (h0 + 16 * g) // 16
                        # NOTE: alternating (mains,leftovers)/(leftovers,
                        # mains) order to halve PE mode switches is UNSAFE:
                        # starting a bank on one quadrant matmul clears the
                        # whole bank while the 4ns-later sibling quadrant is
                        # writing it (hardware error).  Keep mains first.
                        rev = False
                        gidx += 1
                        last_g = b == bpc - 1 and blk == nblk - 1 and g == ngrp - 1
                        st1 = spool.tile([128, 4, 2, W], odt, tag="st1")
                        st2 = spool.tile([128, 4, 2, W], odt, tag="st2")
                        # bank1(uu) = (E1 | O1) rows, bank2(uu) = (O2 | E2)
                        ps = [
                            (
                                ppool.tile([128, 2, W], f32, tag="ps1", name="ps1"),
                                ppool.tile([128, 2, W], f32, tag="ps2", name="ps2"),
                            )
                            for _ in range(4)
                        ]

                        def mains(first):
                            # fused mains for all four uu: E rows (kh=1,2) +
                            # O rows (kh=0,1) share the rhs stream -> one
                            # M=128 full-array matmul per (uu, pair, kw).
                            for uu in range(4):
                                s0 = 16 * g + 4 * uu + 1  # A-chunk of x row r0
                                ps1, ps2 = ps[uu]
                                for kw in range(3):
                                    st_ = first and kw == 0
                                    sp_ = (not first) and kw == 2
                                    nc.tensor.matmul(
                                        ps1[0:128], wt[:, kw, :],
                                        xt[:, s0 : s0 + 2, kw : kw + W],
                                        start=st_, stop=sp_, skip_group_check=rev,
                                    )
                                    nc.tensor.matmul(
                                        ps2[0:128], wt[:, 3 + kw, :],
                                        xt[:, s0 + 2 : s0 + 4, kw : kw + W],
                                        start=st_, stop=sp_, skip_group_check=rev,
                                    )
                                if not first:
                                    evac(uu)

                        def leftovers(first):
                            # K=64 leftovers: E tap kh=0 from half B, O tap
                            # kh=2 from half A; per kw the four land on
                            # disjoint quadrants (64,0) (0,64) (64,64) (0,0)
                            for uu in range(4):
                                s0 = 16 * g + 4 * uu + 1
                                ps1, ps2 = ps[uu]
                                for kw in range(3):
                                    st_ = first and kw == 0
                                    sp_ = (not first) and kw == 2
                                    nc.tensor.matmul(
                                        ps1[0:64], wt[64:128, 6 + kw, 0:64],
                                        xt[64:128, s0 - 1 : s0 + 1, kw : kw + W],
                                        start=st_, stop=sp_, skip_group_check=rev,
                                    )
                                    nc.tensor.matmul(
                                        ps1[64:128], wt[0:64, 6 + kw, 0:64],
                                        xt[0:64, s0 + 1 : s0 + 3, kw : kw + W],
                                        start=False, stop=sp_, skip_group_check=rev,
                                    )
                                    nc.tensor.matmul(
                                        ps2[64:128], wt[64:128, 6 + kw, 64:128],
                                        xt[64:128, s0 + 1 : s0 + 3, kw : kw + W],
                                        start=st_, stop=sp_, skip_group_check=rev,
                                    )
                                    nc.tensor.matmul(
                                        ps2[0:64], wt[0:64, 6 + kw, 64:128],
                                        xt[0:64, s0 + 3 : s0 + 5, kw : kw + W],
                                        start=False, stop=sp_, skip_group_check=rev,
                                    )
                                if not first:
                                    evac(uu)

                        def evac(uu):
                            ps1, ps2 = ps[uu]
                            nc.scalar.copy(st1[:, uu], ps1[:])
                            nc.vector.tensor_copy(out=st2[:, uu], in_=ps2[:])
                            if last_g:
                                # stream the final group per-uu across both
                                # HWDGE queues (sync + scalar) so issue
                                # latency overlaps the remaining compute
                                nc.sync.dma_start(
                                    out=out.ap()[b, 0, :, hg, uu], in_=st1[:, uu]
                                )
                                nc.scalar.dma_start(
                                    out=out.ap()[b, 1, :, hg, uu], in_=st2[:, uu]
                                )

                        if rev:
                            leftovers(first=True)
                            mains(first=False)
                        else:
                            mains(first=True)
                            leftovers(first=False)

                        if not last_g:
                            # group output DMAs on gpsimd: sync stays free
                            # to prefetch the next block's x tiles
                            nc.gpsimd.dma_start(out=out.ap()[b, 0, :, hg], in_=st1[:])
                            nc.gpsimd.dma_start(out=out.ap()[b, 1, :, hg], in_=st2[:])
    nc.compile()
    return nc


def normalize_weight(weight):
    """Host-side equalized-lr + demodulation of the [O,I,3,3] weight."""
    w = np.asarray(weight, dtype=np.float32) * np.float32(C_EQ)
    sigma_inv = 1.0 / np.sqrt(
        np.sum((w * w).astype(np.float32), axis=(1, 2, 3), keepdims=True) + EPS
    )
    return (w * sigma_inv.astype(np.float32)).astype(np.float32)


def pack_weights(w_norm):
    """Pack normalized [O,I,kh,kw] weights into the [128, 9, 128] SBUF image.

    Slot kw (0..2) is the fused main weight for row-pair 1: cols 0:64 are
    the even-row mains (rows 0:64 <- kh=1, rows 64:128 <- kh=2), cols
    64:128 the odd-row mains (kh=0 / kh=1).  Slot 3+kw is the same for
    row-pair 2 with the column halves swapped (O | E).  Slot 6+kw holds
    the K=64 leftovers: cols 0:64 rows 0:64 <- kh=2 (O tap), rows 64:128
    <- kh=0 (E tap); cols 64:128 duplicate them for the second row-pair's
    quadrants.  Each [64, 64] sub-block is w[:, :, kh, kw].T (contraction
    dim first).
    """
    wt = np.transpose(w_norm, (2, 3, 1, 0))  # [kh, kw, in, out]
    wpack = np.zeros((128, 9, 128), dtype=np.float32)
    for kw in range(3):
        # fused mains, row-pair 1: [E | O]
        wpack[0:64, kw, 0:64] = wt[1, kw]
        wpack[64:128, kw, 0:64] = wt[2, kw]
        wpack[0:64, kw, 64:128] = wt[0, kw]
        wpack[64:128, kw, 64:128] = wt[1, kw]
        # fused mains, row-pair 2: [O | E]
        wpack[0:64, 3 + kw, 0:64] = wt[0, kw]
        wpack[64:128, 3 + kw, 0:64] = wt[1, kw]
        wpack[0:64, 3 + kw, 64:128] = wt[1, kw]
        wpack[64:128, 3 + kw, 64:128] = wt[2, kw]
        # leftovers (both col-halves identical)
        for half in (0, 64):
            wpack[0:64, 6 + kw, half : half + 64] = wt[2, kw]
            wpack[64:128, 6 + kw, half : half + 64] = wt[0, kw]
    return wpack


_NC_CACHE = {}


def _get_nc(bpc, h, block=64, out_bf16=True):
    key = (bpc, h, block, out_bf16)
    if key not in _NC_CACHE:
        _NC_CACHE[key] = build_nc(bpc, h, block, out_bf16)
    return _NC_CACHE[key]


def split_parity(x_f32):
    """[b, c, h, w] f32 -> bf16 [b, 2*c, h//2, w+2]: row parity split plus
    zero border columns; partition k = 64*p + c (p=0 even rows, p=1 odd)."""
    import ml_dtypes

    b, c, h, w = x_f32.shape
    xb = x_f32.astype(ml_dtypes.bfloat16)
    xP = np.zeros((b, 2, c, h // 2, w + 2), dtype=ml_dtypes.bfloat16)
    xP[:, 0, :, :, 1:-1] = xb[:, :, 0::2]
    xP[:, 1, :, :, 1:-1] = xb[:, :, 1::2]
    return xP.reshape(b, 2 * c, h // 2, w + 2)


def merge_parity(outP):
    """[b, 2, 128, hgs, 4, 2, w] (any float dtype) -> fp32 [b, 64, h, w]
    with partition k = 64*ph + c, parity p = ph ^ uh and image row
    2*(16*hg + 4*uu + 2*uh + up) + p = 32*hg + 8*uu + 4*uh + 2*up + p."""
    b, _, _, hgs, _, _, w = outP.shape
    o = np.asarray(outP, dtype=np.float32).reshape(b, 2, 2, 64, hgs, 4, 2, w)
    full = np.empty((b, 64, hgs, 4, 2, 2, 2, w), dtype=np.float32)
    for uh in range(2):
        for ph in range(2):
            full[:, :, :, :, uh, :, ph ^ uh, :] = o[:, uh, ph]
    return full.reshape(b, 64, 32 * hgs, w)


def kernel(x, weight):
    import ml_dtypes
    from concourse import bass_utils

    x = np.asarray(x, dtype=np.float32)
    weight = np.asarray(weight, dtype=np.float32)
    assert x.shape == (B_FULL, IN_F, H_FULL, W), x.shape

    xP = split_parity(x)
    wpack = pack_weights(normalize_weight(weight)).astype(ml_dtypes.bfloat16)
    bpc = B_FULL // N_CORES
    nc = _get_nc(bpc, H_FULL)
    in_maps = [
        {"x": xP[i * bpc : (i + 1) * bpc], "wpack": wpack} for i in range(N_CORES)
    ]
    res = bass_utils.run_bass_kernel_spmd(nc, in_maps, core_ids=list(range(N_CORES)))
    return np.concatenate([merge_parity(r["out"]) for r in res.results], axis=0)


# revision 9
# speedup vs baseline: 1.2153x; 1.0075x over previous
"""Trainium2 Bass kernel for Conv2dWeightModulate (no style).

The reference computes an equalized-lr + demodulated 3x3 conv:
    w = weight * C_EQ;  w *= rsqrt(sum(w^2, (I,K,K)) + eps);  out = conv2d(x, w, pad=1)

The tiny weight normalization runs on host (numpy); the conv runs on 8
NeuronCores, data-parallel over the batch (2 images per core).

Host-side data layout: x is cast to bf16 and split by row parity into
xP[b, 128, h2, w+2] where partition k = 64*p + c holds x[b, c, 2*h2+p, w]
(p = row parity), rows pre-padded to 258 columns with zero borders.  Both
parity halves share the same h2 indexing, so each block's SBUF x tile
loads with single full-128-partition DMAs (all 16 SDMA engines engaged).

Device kernel layout (per core):
  x is stored in SBUF parity-interleaved: partitions 0-63 hold the 64
  channels of even image rows, partitions 64-127 the odd rows.  Chunk
  column s of a block with row base R holds h2 row R/2 - 1 + s of both
  halves, so chunk s aligns x rows (2j, 2j+1) vertically.  Per 8 output
  rows (free dim 512 = 2 rows x 256), the 3x3 conv is:
      - mains: even rows (taps kh=1,2) and odd rows (taps kh=0,1) read the
        SAME rhs chunks, so they fuse into one full-array M=128 matmul
        (E outputs in one 64-col group, O in the other) x3 kw x2 row-pairs.
      - leftovers: the third tap of each parity is K=64; the four of them
        (E1,O1,E2,O2) land on the four disjoint PE quadrants per kw and
        run concurrently.
  Switching the PE between the full-array mains and the quadrant
  leftovers costs a full-array drain (~90ns over a slot); alternating
  the phase order per group to halve the switches is NOT safe: a
  start=True bank clear while the sibling quadrant matmul writes the
  same bank is a hardware error (NRT_EXEC_UNIT_UNRECOVERABLE).
  Bank 1 holds (E1|O1) rows, bank 2 (O2|E2) so the quadrants stay
  disjoint; each bank evacuates with a single 128-partition copy (ACT for
  bank 1, DVE for bank 2 - never the same bank).  Accumulation is fp32 in
  PSUM; outputs stage through SBUF as bf16 and DMA out as bf16 with a
  single 128-partition store per staging tile into
  out[b, uh, k, hg, uu, up, w] where partition k = 64*ph + c, row parity
  p = ph ^ uh and h2 = 16*hg + 4*uu + 2*uh + up; every partition's store
  is one contiguous 4 KiB span.  The final group streams per-uu split
  over the sync+scalar HWDGE queues so the kernel does not end on a
  large store drain.
  Startup: the mains weights + first x chunks stream on sync while the
  leftover weights + later chunks go via scalar, and three throwaway
  matmuls on a zeroed scratch tile start the PE HAM warmup window while
  the first x chunks are still in flight.
"""

import numpy as np

IN_F = 64
OUT_F = 64
KS = 3
EPS = 1e-05
C_EQ = 1.0 / np.sqrt(IN_F * KS * KS)

B_FULL = 16
H_FULL = 256
W = 256
N_CORES = 8
CW = W + 2  # padded row width


def build_nc(bpc, h, block=64, out_bf16=True):
    """Build the per-core Bass program: bpc images of [64, h, 256] each."""
    from concourse import bacc
    import concourse.mybir as mybir
    from concourse.tile import TileContext

    assert h % block == 0 and block % 32 == 0
    nblk = h // block
    ngrp = block // 32  # 32-row output staging groups per block
    sch = block // 2 + 2  # chunk columns per x tile
    hgs = h // 32  # 32-row output groups per image
    f32 = mybir.dt.float32
    bf16 = mybir.dt.bfloat16

    nc = bacc.Bacc("TRN2", target_bir_lowering=False, debug=False)
    # partition k = 64*p + c  (p = row parity, c = channel)
    x = nc.dram_tensor("x", [bpc, 2 * IN_F, h // 2, CW], bf16, kind="ExternalInput")
    wp = nc.dram_tensor("wpack", [128, 9, 128], bf16, kind="ExternalInput")
    odt = bf16 if out_bf16 else f32
    # out[b, uh, k, hg, uu, up, w]: partition k = 64*ph + c, parity
    # p = ph ^ uh, h2 = 16*hg + 4*uu + 2*uh + up, image row = 2*h2 + p.
    out = nc.dram_tensor(
        "out", [bpc, 2, 128, hgs, 4, 2, W], odt, kind="ExternalOutput"
    )

    with TileContext(nc) as tc:
        with (
            tc.tile_pool(name="xp", bufs=4) as xpool,
            tc.tile_pool(name="wpool", bufs=1) as wpool,
            tc.tile_pool(name="wm", bufs=1) as wmpool,
            tc.tile_pool(name="st", bufs=3) as spool,
            tc.tile_pool(name="ps", bufs=4, space="PSUM") as ppool,
        ):
            wt = wpool.tile([128, 9, 128], bf16)
            # mains weights (slots 0-5) via sync; leftovers via scalar (the
            # scalar queue opens later, behind the preamble ACT_TABLE_LOAD)
            nc.sync.dma_start(out=wt[:, 0:6, :], in_=wp.ap()[:, 0:6, :])
            nc.scalar.dma_start(out=wt[:, 6:9, :], in_=wp.ap()[:, 6:9, :])

            # PE warmup: start the HAM activity window while the first x
            # chunks are still in flight (results are discarded).  The
            # warmup matmuls alternate PSUM banks: a start=True bank clear
            # must never fire while the previous matmul is still draining
            # into the same bank.
            wmt = wmpool.tile([128, 2, W], bf16)
            nc.gpsimd.memset(wmt[:], 0.0)
            pw1 = ppool.tile([128, 2, W], f32, tag="ps1", name="psw1")
            pw2 = ppool.tile([128, 2, W], f32, tag="ps2", name="psw2")
            # ~9 cold matmuls span the ~4us worst-case wait for the first x
            # chunks (SDMA engine wake jitter) and flip HAM to full clock
            # before the real stream begins; if the DMA lands early the
            # first real matmuls queue right behind these at worst ~2us
            # later but run warm instead of half-clock, a wash.
            for i in range(9):
                nc.tensor.matmul(
                    (pw1 if i % 2 == 0 else pw2)[:], wmt[:, 0, 0:128], wmt[:],
                    start=True, stop=True,
                )

            gidx = 0
            for b in range(bpc):
                for blk in range(nblk):
                    R = blk * block
                    h0 = R // 2
                    xt = xpool.tile([128, sch, CW], bf16, tag="xt")
                    # chunk s of both halves <- h2 row (h0-1)+s; boundary
                    # chunks that fall outside the image are either zeroed
                    # (read as conv padding) or skipped (never read).
                    if blk == 0:
                        c_lo, c_hi = 1, sch  # A chunk 0 unused; B chunk 0 zero
                        nc.gpsimd.memset(xt[64:128, 0, :], 0.0)
                    elif blk == nblk - 1:
                        c_lo, c_hi = 0, sch - 1  # B chunk sch-1 unused
                        nc.gpsimd.memset(xt[0:64, sch - 1, :], 0.0)
                    else:
                        c_lo, c_hi = 0, sch
                    r_lo = h0 - 1 + c_lo
                    # the very first tile streams in small pieces so compute
                    # starts as soon as the first chunks land.  ALL x pieces
                    # stay on the sync HWDGE ring: one ring completes FIFO,
                    # so the early chunks finish first; a second ring's
                    # packets would round-robin with these on the 16 SDMA
                    # engines and starve the critical first pieces.
                    if b == 0 and blk == 0:
                        pieces = [(1, 3, 0), (3, 6, 0), (6, 10, 0), (10, 16, 0),
                                  (16, 23, 0), (23, 34, 0)]
                    else:
                        mid = (c_lo + c_hi) // 2
                        pieces = [(c_lo, mid, 0), (mid, c_hi, 0)]
                    for s_lo, s_hi, eng in pieces:
                        e = nc.scalar if eng else nc.sync
                        e.dma_start(
                            out=xt[:, s_lo:s_hi, :],
                            in_=x.ap()[b, :, s_lo - c_lo + r_lo : s_hi - c_lo + r_lo, :],
                        )
                    for g in range(ngrp):
                        hg = (h0 + 16 * g) // 16
                        # NOTE: alternating (mains,leftovers)/(leftovers,
                        # mains) order to halve PE mode switches is UNSAFE:
                        # starting a bank on one quadrant matmul clears the
                        # whole bank while the 4ns-later sibling quadrant is
                        # writing it (hardware error).  Keep mains first.
                        rev = False
                        gidx += 1
                        last_g = b == bpc - 1 and blk == nblk - 1 and g == ngrp - 1
                        st1 = spool.tile([128, 4, 2, W], odt, tag="st1")
                        st2 = spool.tile([128, 4, 2, W], odt, tag="st2")
                        # bank1(uu) = (E1 | O1) rows, bank2(uu) = (O2 | E2)
                        ps = [
                            (
                                ppool.tile([128, 2, W], f32, tag="ps1", name="ps1"),
                                ppool.tile([128, 2, W], f32, tag="ps2", name="ps2"),
                            )
                            for _ in range(4)
                        ]

                        def mains(first):
                            # fused mains for all four uu: E rows (kh=1,2) +
                            # O rows (kh=0,1) share the rhs stream -> one
                            # M=128 full-array matmul per (uu, pair, kw).
                            for uu in range(4):
                                s0 = 16 * g + 4 * uu + 1  # A-chunk of x row r0
                                ps1, ps2 = ps[uu]
                                for kw in range(3):
                                    st_ = first and kw == 0
                                    sp_ = (not first) and kw == 2
                                    nc.tensor.matmul(
                                        ps1[0:128], wt[:, kw, :],
                                        xt[:, s0 : s0 + 2, kw : kw + W],
                                        start=st_, stop=sp_, skip_group_check=rev,
                                    )
                                    nc.tensor.matmul(
                                        ps2[0:128], wt[:, 3 + kw, :],
                                        xt[:, s0 + 2 : s0 + 4, kw : kw + W],
                                        start=st_, stop=sp_, skip_group_check=rev,
                                    )
                                if not first:
                                    evac(uu)

                        def leftovers(first):
                            # K=64 leftovers: E tap kh=0 from half B, O tap
                            # kh=2 from half A; per kw the four land on
                            # disjoint quadrants (64,0) (0,64) (64,64) (0,0)
                            for uu in range(4):
                                s0 = 16 * g + 4 * uu + 1
                                ps1, ps2 = ps[uu]
                                for kw in range(3):
                                    st_ = first and kw == 0
                                    sp_ = (not first) and kw == 2
                                    nc.tensor.matmul(
                                        ps1[0:64], wt[64:128, 6 + kw, 0:64],
                                        xt[64:128, s0 - 1 : s0 + 1, kw : kw + W],
                                        start=st_, stop=sp_, skip_group_check=rev,
                                    )
                                    nc.tensor.matmul(
                                        ps1[64:128], wt[0:64, 6 + kw, 0:64],
                                        xt[0:64, s0 + 1 : s0 + 3, kw : kw + W],
                                        start=False, stop=sp_, skip_group_check=rev,
                                    )
                                    nc.tensor.matmul(
                                        ps2[64:128], wt[64:128, 6 + kw, 64:128],
                                        xt[64:128, s0 + 1 : s0 + 3, kw : kw + W],
                                        start=st_, stop=sp_, skip_group_check=rev,
                                    )
                                    nc.tensor.matmul(
                                        ps2[0:64], wt[0:64, 6 + kw, 64:128],
                                        xt[0:64, s0 + 3 : s0 + 5, kw : kw + W],
                                        start=False, stop=sp_, skip_group_check=rev,
                                    )
                                if not first:
                                    evac(uu)

                        def evac(uu):
                            ps1, ps2 = ps[uu]
                            nc.scalar.copy(st1[:, uu], ps1[:])
                            nc.vector.tensor_copy(out=st2[:, uu], in_=ps2[:])
                            if last_g:
                                # stream the final group per-uu across both
                                # HWDGE queues (sync + scalar) so issue
                                # latency overlaps the remaining compute
                                nc.sync.dma_start(
                                    out=out.ap()[b, 0, :, hg, uu], in_=st1[:, uu]
                                )
                                nc.scalar.dma_start(
                                    out=out.ap()[b, 1, :, hg, uu], in_=st2[:, uu]
                                )

                        if rev:
                            leftovers(first=True)
                            mains(first=False)
                        else:
                            mains(first=True)
                            leftovers(first=False)

                        if not last_g:
                            # group output DMAs on gpsimd: sync stays free
                            # to prefetch the next block's x tiles
                            nc.gpsimd.dma_start(out=out.ap()[b, 0, :, hg], in_=st1[:])
                            nc.gpsimd.dma_start(out=out.ap()[b, 1, :, hg], in_=st2[:])
    nc.compile()
    return nc


def normalize_weight(weight):
    """Host-side equalized-lr + demodulation of the [O,I,3,3] weight."""
    w = np.asarray(weight, dtype=np.float32) * np.float32(C_EQ)
    sigma_inv = 1.0 / np.sqrt(
        np.sum((w * w).astype(np.float32), axis=(1, 2, 3), keepdims=True) + EPS
    )
    return (w * sigma_inv.astype(np.float32)).astype(np.float32)


def pack_weights(w_norm):
    """Pack normalized [O,I,kh,kw] weights into the [128, 9, 128] SBUF image.

    Slot kw (0..2) is the fused main weight for row-pair 1: cols 0:64 are
    the even-row mains (rows 0:64 <- kh=1, rows 64:128 <- kh=2), cols
    64:128 the odd-row mains (kh=0 / kh=1).  Slot 3+kw is the same for
    row-pair 2 with the column halves swapped (O | E).  Slot 6+kw holds
    the K=64 leftovers: cols 0:64 rows 0:64 <- kh=2 (O tap), rows 64:128
    <- kh=0 (E tap); cols 64:128 duplicate them for the second row-pair's
    quadrants.  Each [64, 64] sub-block is w[:, :, kh, kw].T (contraction
    dim first).
    """
    wt = np.transpose(w_norm, (2, 3, 1, 0))  # [kh, kw, in, out]
    wpack = np.zeros((128, 9, 128), dtype=np.float32)
    for kw in range(3):
        # fused mains, row-pair 1: [E | O]
        wpack[0:64, kw, 0:64] = wt[1, kw]
        wpack[64:128, kw, 0:64] = wt[2, kw]
        wpack[0:64, kw, 64:128] = wt[0, kw]
        wpack[64:128, kw, 64:128] = wt[1, kw]
        # fused mains, row-pair 2: [O | E]
        wpack[0:64, 3 + kw, 0:64] = wt[0, kw]
        wpack[64:128, 3 + kw, 0:64] = wt[1, kw]
        wpack[0:64, 3 + kw, 64:128] = wt[1, kw]
        wpack[64:128, 3 + kw, 64:128] = wt[2, kw]
        # leftovers (both col-halves identical)
        for half in (0, 64):
            wpack[0:64, 6 + kw, half : half + 64] = wt[2, kw]
            wpack[64:128, 6 + kw, half : half + 64] = wt[0, kw]
    return wpack


_NC_CACHE = {}


def _get_nc(bpc, h, block=64, out_bf16=True):
    key = (bpc, h, block, out_bf16)
    if key not in _NC_CACHE:
        _NC_CACHE[key] = build_nc(bpc, h, block, out_bf16)
    return _NC_CACHE[key]


def split_parity(x_f32):
    """[b, c, h, w] f32 -> bf16 [b, 2*c, h//2, w+2]: row parity split plus
    zero border columns; partition k = 64*p + c (p=0 even rows, p=1 odd)."""
    import ml_dtypes

    b, c, h, w = x_f32.shape
    xb = x_f32.astype(ml_dtypes.bfloat16)
    xP = np.zeros((b, 2, c, h // 2, w + 2), dtype=ml_dtypes.bfloat16)
    xP[:, 0, :, :, 1:-1] = xb[:, :, 0::2]
    xP[:, 1, :, :, 1:-1] = xb[:, :, 1::2]
    return xP.reshape(b, 2 * c, h // 2, w + 2)


def merge_parity(outP):
    """[b, 2, 128, hgs, 4, 2, w] (any float dtype) -> fp32 [b, 64, h, w]
    with partition k = 64*ph + c, parity p = ph ^ uh and image row
    2*(16*hg + 4*uu + 2*uh + up) + p = 32*hg + 8*uu + 4*uh + 2*up + p."""
    b, _, _, hgs, _, _, w = outP.shape
    o = np.asarray(outP, dtype=np.float32).reshape(b, 2, 2, 64, hgs, 4, 2, w)
    full = np.empty((b, 64, hgs, 4, 2, 2, 2, w), dtype=np.float32)
    for uh in range(2):
        for ph in range(2):
            full[:, :, :, :, uh, :, ph ^ uh, :] = o[:, uh, ph]
    return full.reshape(b, 64, 32 * hgs, w)


def kernel(x, weight):
    import ml_dtypes
    from concourse import bass_utils

    x = np.asarray(x, dtype=np.float32)
    weight = np.asarray(weight, dtype=np.float32)
    assert x.shape == (B_FULL, IN_F, H_FULL, W), x.shape

    xP = split_parity(x)
    wpack = pack_weights(normalize_weight(weight)).astype(ml_dtypes.bfloat16)
    bpc = B_FULL // N_CORES
    nc = _get_nc(bpc, H_FULL)
    in_maps = [
        {"x": xP[i * bpc : (i + 1) * bpc], "wpack": wpack} for i in range(N_CORES)
    ]
    res = bass_utils.run_bass_kernel_spmd(nc, in_maps, core_ids=list(range(N_CORES)))
    return np.concatenate([merge_parity(r["out"]) for r in res.results], axis=0)


# revision 13
# speedup vs baseline: 1.2234x; 1.0067x over previous
"""Trainium2 Bass kernel for Conv2dWeightModulate (no style).

The reference computes an equalized-lr + demodulated 3x3 conv:
    w = weight * C_EQ;  w *= rsqrt(sum(w^2, (I,K,K)) + eps);  out = conv2d(x, w, pad=1)

The tiny weight normalization runs on host (numpy); the conv runs on 8
NeuronCores, data-parallel over the batch (2 images per core).

Host-side data layout: x is cast to bf16 and split by row parity into
xP[b, 128, h2, w+2] where partition k = 64*p + c holds x[b, c, 2*h2+p, w]
(p = row parity), rows pre-padded to 258 columns with zero borders.  Both
parity halves share the same h2 indexing, so each block's SBUF x tile
loads with single full-128-partition DMAs (all 16 SDMA engines engaged).

Device kernel layout (per core):
  x is stored in SBUF parity-interleaved: partitions 0-63 hold the 64
  channels of even image rows, partitions 64-127 the odd rows.  Chunk
  column s of a block with row base R holds h2 row R/2 - 1 + s of both
  halves, so chunk s aligns x rows (2j, 2j+1) vertically.  Per 8 output
  rows (free dim 512 = 2 rows x 256), the 3x3 conv is:
      - mains: even rows (taps kh=1,2) and odd rows (taps kh=0,1) read the
        SAME rhs chunks, so they fuse into one full-array M=128 matmul
        (E outputs in one 64-col group, O in the other) x3 kw x2 row-pairs.
      - leftovers: the third tap of each parity is K=64; the four of them
        (E1,O1,E2,O2) land on the four disjoint PE quadrants per kw and
        run concurrently.
  Switching the PE between the full-array mains and the quadrant
  leftovers costs a full-array drain (~90ns over a slot); alternating
  the phase order per group to halve the switches is NOT safe: a
  start=True bank clear while the sibling quadrant matmul writes the
  same bank is a hardware error (NRT_EXEC_UNIT_UNRECOVERABLE).
  Bank 1 holds (E1|O1) rows, bank 2 (O2|E2) so the quadrants stay
  disjoint; each bank evacuates with a single 128-partition copy (ACT for
  bank 1, DVE for bank 2 - never the same bank).  Accumulation is fp32 in
  PSUM; outputs stage through SBUF as bf16 and DMA out as bf16 with a
  single 128-partition store per staging tile into
  out[b, uh, k, hg, uu, up, w] where partition k = 64*ph + c, row parity
  p = ph ^ uh and h2 = 16*hg + 4*uu + 2*uh + up; every partition's store
  is one contiguous 4 KiB span.  The final group streams per-uu split
  over the sync+scalar HWDGE queues so the kernel does not end on a
  large store drain.
  Startup: the mains weights + all x chunks stream in order on the sync
  ring (leftover weights via scalar), and nine throwaway matmuls on a
  zeroed scratch tile ride out the SDMA wake jitter and start the PE HAM
  warmup window while the first x chunks are still in flight.
"""

import numpy as np

IN_F = 64
OUT_F = 64
KS = 3
EPS = 1e-05
C_EQ = 1.0 / np.sqrt(IN_F * KS * KS)

B_FULL = 16
H_FULL = 256
W = 256
N_CORES = 8
CW = W + 2  # padded row width


def build_nc(bpc, h, block=64, out_bf16=True):
    """Build the per-core Bass program: bpc images of [64, h, 256] each."""
    from concourse import bacc
    import concourse.mybir as mybir
    from concourse.tile import TileContext

    assert h % block == 0 and block % 32 == 0
    nblk = h // block
    ngrp = block // 32  # 32-row output staging groups per block
    sch = block // 2 + 2  # chunk columns per x tile
    hgs = h // 32  # 32-row output groups per image
    f32 = mybir.dt.float32
    bf16 = mybir.dt.bfloat16

    nc = bacc.Bacc("TRN2", target_bir_lowering=False, debug=False)
    # partition k = 64*p + c  (p = row parity, c = channel)
    x = nc.dram_tensor("x", [bpc, 2 * IN_F, h // 2, CW], bf16, kind="ExternalInput")
    wp = nc.dram_tensor("wpack", [128, 9, 128], bf16, kind="ExternalInput")
    odt = bf16 if out_bf16 else f32
    # out[b, uh, k, hg, uu, up, w]: partition k = 64*ph + c, parity
    # p = ph ^ uh, h2 = 16*hg + 4*uu + 2*uh + up, image row = 2*h2 + p.
    out = nc.dram_tensor(
        "out", [bpc, 2, 128, hgs, 4, 2, W], odt, kind="ExternalOutput"
    )

    with TileContext(nc) as tc:
        with (
            tc.tile_pool(name="xp", bufs=4) as xpool,
            tc.tile_pool(name="wpool", bufs=1) as wpool,
            tc.tile_pool(name="wm", bufs=1) as wmpool,
            tc.tile_pool(name="st", bufs=3) as spool,
            tc.tile_pool(name="ps", bufs=4, space="PSUM") as ppool,
        ):
            wt = wpool.tile([128, 9, 128], bf16)
            # mains weights (slots 0-5) via sync; leftovers via scalar (the
            # scalar queue opens later, behind the preamble ACT_TABLE_LOAD)
            nc.sync.dma_start(out=wt[:, 0:6, :], in_=wp.ap()[:, 0:6, :])
            nc.scalar.dma_start(out=wt[:, 6:9, :], in_=wp.ap()[:, 6:9, :])

            # PE warmup: start the HAM activity window while the first x
            # chunks are still in flight (results are discarded).  The
            # warmup matmuls alternate PSUM banks: a start=True bank clear
            # must never fire while the previous matmul is still draining
            # into the same bank.
            wmt = wmpool.tile([128, 2, W], bf16)
            nc.gpsimd.memset(wmt[:], 0.0)
            pw1 = ppool.tile([128, 2, W], f32, tag="ps1", name="psw1")
            pw2 = ppool.tile([128, 2, W], f32, tag="ps2", name="psw2")
            # ~9 cold matmuls span the ~4us worst-case wait for the first x
            # chunks (SDMA engine wake jitter) and flip HAM to full clock
            # before the real stream begins; if the DMA lands early the
            # first real matmuls queue right behind these at worst ~2us
            # later but run warm instead of half-clock, a wash.
            for i in range(9):
                nc.tensor.matmul(
                    (pw1 if i % 2 == 0 else pw2)[:], wmt[:, 0, 0:128], wmt[:],
                    start=True, stop=True,
                )

            gidx = 0
            for b in range(bpc):
                for blk in range(nblk):
                    R = blk * block
                    h0 = R // 2
                    xt = xpool.tile([128, sch, CW], bf16, tag="xt")
                    # chunk s of both halves <- h2 row (h0-1)+s; boundary
                    # chunks that fall outside the image are either zeroed
                    # (read as conv padding) or skipped (never read).
                    if blk == 0:
                        c_lo, c_hi = 1, sch  # A chunk 0 unused; B chunk 0 zero
                        nc.gpsimd.memset(xt[64:128, 0, :], 0.0)
                    elif blk == nblk - 1:
                        c_lo, c_hi = 0, sch - 1  # B chunk sch-1 unused
                        nc.gpsimd.memset(xt[0:64, sch - 1, :], 0.0)
                    else:
                        c_lo, c_hi = 0, sch
                    r_lo = h0 - 1 + c_lo
                    # the very first tile streams in small pieces so compute
                    # starts as soon as the first chunks land.  ALL x pieces
                    # stay on the sync HWDGE ring: one ring completes FIFO,
                    # so the early chunks finish first; a second ring's
                    # packets would round-robin with these on the 16 SDMA
                    # engines and starve the critical first pieces.
                    if b == 0 and blk == 0:
                        pieces = [(1, 3, 0), (3, 6, 0), (6, 10, 0), (10, 16, 0),
                                  (16, 23, 0), (23, 34, 0)]
                    else:
                        mid = (c_lo + c_hi) // 2
                        pieces = [(c_lo, mid, 0), (mid, c_hi, 0)]
                    for s_lo, s_hi, eng in pieces:
                        e = nc.scalar if eng else nc.sync
                        e.dma_start(
                            out=xt[:, s_lo:s_hi, :],
                            in_=x.ap()[b, :, s_lo - c_lo + r_lo : s_hi - c_lo + r_lo, :],
                        )
                    for g in range(ngrp):
                        hg = (h0 + 16 * g) // 16
                        # NOTE: alternating (mains,leftovers)/(leftovers,
                        # mains) order to halve PE mode switches is UNSAFE:
                        # starting a bank on one quadrant matmul clears the
                        # whole bank while the 4ns-later sibling quadrant is
                        # writing it (hardware error).  Keep mains first.
                        rev = False
                        gidx += 1
                        last_g = b == bpc - 1 and blk == nblk - 1 and g == ngrp - 1
                        st1 = spool.tile([128, 4, 2, W], odt, tag="st1")
                        st2 = spool.tile([128, 4, 2, W], odt, tag="st2")
                        # bank1(uu) = (E1 | O1) rows, bank2(uu) = (O2 | E2)
                        ps = [
                            (
                                ppool.tile([128, 2, W], f32, tag="ps1", name="ps1"),
                                ppool.tile([128, 2, W], f32, tag="ps2", name="ps2"),
                            )
                            for _ in range(4)
                        ]

                        def mains(first):
                            # fused mains for all four uu: E rows (kh=1,2) +
                            # O rows (kh=0,1) share the rhs stream -> one
                            # M=128 full-array matmul per (uu, pair, kw).
                            for uu in range(4):
                                s0 = 16 * g + 4 * uu + 1  # A-chunk of x row r0
                                ps1, ps2 = ps[uu]
                                for kw in range(3):
                                    st_ = first and kw == 0
                                    sp_ = (not first) and kw == 2
                                    nc.tensor.matmul(
                                        ps1[0:128], wt[:, kw, :],
                                        xt[:, s0 : s0 + 2, kw : kw + W],
                                        start=st_, stop=sp_, skip_group_check=rev,
                                    )
                                    nc.tensor.matmul(
                                        ps2[0:128], wt[:, 3 + kw, :],
                                        xt[:, s0 + 2 : s0 + 4, kw : kw + W],
                                        start=st_, stop=sp_, skip_group_check=rev,
                                    )
                                if not first:
                                    evac(uu)

                        def leftovers(first):
                            # K=64 leftovers: E tap kh=0 from half B, O tap
                            # kh=2 from half A; per kw the four land on
                            # disjoint quadrants (64,0) (0,64) (64,64) (0,0)
                            for uu in range(4):
                                s0 = 16 * g + 4 * uu + 1
                                ps1, ps2 = ps[uu]
                                for kw in range(3):
                                    st_ = first and kw == 0
                                    sp_ = (not first) and kw == 2
                                    nc.tensor.matmul(
                                        ps1[0:64], wt[64:128, 6 + kw, 0:64],
                                        xt[64:128, s0 - 1 : s0 + 1, kw : kw + W],
                                        start=st_, stop=sp_, skip_group_check=rev,
                                    )
                                    nc.tensor.matmul(
                                        ps1[64:128], wt[0:64, 6 + kw, 0:64],
                                        xt[0:64, s0 + 1 : s0 + 3, kw : kw + W],
                                        start=False, stop=sp_, skip_group_check=rev,
                                    )
                                    nc.tensor.matmul(
                                        ps2[64:128], wt[64:128, 6 + kw, 64:128],
                                        xt[64:128, s0 + 1 : s0 + 3, kw : kw + W],
                                        start=st_, stop=sp_, skip_group_check=rev,
                                    )
                                    nc.tensor.matmul(
                                        ps2[0:64], wt[0:64, 6 + kw, 64:128],
                                        xt[0:64, s0 + 3 : s0 + 5, kw : kw + W],
                                        start=False, stop=sp_, skip_group_check=rev,
                                    )
                                if not first:
                                    evac(uu)

                        def evac(uu):
                            ps1, ps2 = ps[uu]
                            nc.scalar.copy(st1[:, uu], ps1[:])
                            nc.vector.tensor_copy(out=st2[:, uu], in_=ps2[:])
                            if last_g:
                                # stream the final group per-uu across both
                                # HWDGE queues (sync + scalar) so issue
                                # latency overlaps the remaining compute
                                nc.sync.dma_start(
                                    out=out.ap()[b, 0, :, hg, uu], in_=st1[:, uu]
                                )
                                nc.scalar.dma_start(
                                    out=out.ap()[b, 1, :, hg, uu], in_=st2[:, uu]
                                )

                        if rev:
                            leftovers(first=True)
                            mains(first=False)
                        else:
                            mains(first=True)
                            leftovers(first=False)

                        if not last_g:
                            # group output DMAs on gpsimd: sync stays free
                            # to prefetch the next block's x tiles
                            nc.gpsimd.dma_start(out=out.ap()[b, 0, :, hg], in_=st1[:])
                            nc.gpsimd.dma_start(out=out.ap()[b, 1, :, hg], in_=st2[:])
    nc.compile()
    return nc


def normalize_weight(weight):
    """Host-side equalized-lr + demodulation of the [O,I,3,3] weight."""
    w = np.asarray(weight, dtype=np.float32) * np.float32(C_EQ)
    sigma_inv = 1.0 / np.sqrt(
        np.sum((w * w).astype(np.float32), axis=(1, 2, 3), keepdims=True) + EPS
    )
    return (w * sigma_inv.astype(np.float32)).astype(np.float32)


def pack_weights(w_norm):
    """Pack normalized [O,I,kh,kw] weights into the [128, 9, 128] SBUF image.

    Slot kw (0..2) is the fused main weight for row-pair 1: cols 0:64 are
    the even-row mains (rows 0:64 <- kh=1, rows 64:128 <- kh=2), cols
    64:128 the odd-row mains (kh=0 / kh=1).  Slot 3+kw is the same for
    row-pair 2 with the column halves swapped (O | E).  Slot 6+kw holds
    the K=64 leftovers: cols 0:64 rows 0:64 <- kh=2 (O tap), rows 64:128
    <- kh=0 (E tap); cols 64:128 duplicate them for the second row-pair's
    quadrants.  Each [64, 64] sub-block is w[:, :, kh, kw].T (contraction
    dim first).
    """
    wt = np.transpose(w_norm, (2, 3, 1, 0))  # [kh, kw, in, out]
    wpack = np.zeros((128, 9, 128), dtype=np.float32)
    for kw in range(3):
        # fused mains, row-pair 1: [E | O]
        wpack[0:64, kw, 0:64] = wt[1, kw]
        wpack[64:128, kw, 0:64] = wt[2, kw]
        wpack[0:64, kw, 64:128] = wt[0, kw]
        wpack[64:128, kw, 64:128] = wt[1, kw]
        # fused mains, row-pair 2: [O | E]
        wpack[0:64, 3 + kw, 0:64] = wt[0, kw]
        wpack[64:128, 3 + kw, 0:64] = wt[1, kw]
        wpack[0:64, 3 + kw, 64:128] = wt[1, kw]
        wpack[64:128, 3 + kw, 64:128] = wt[2, kw]
        # leftovers (both col-halves identical)
        for half in (0, 64):
            wpack[0:64, 6 + kw, half : half + 64] = wt[2, kw]
            wpack[64:128, 6 + kw, half : half + 64] = wt[0, kw]
    return wpack


_NC_CACHE = {}


def _get_nc(bpc, h, block=64, out_bf16=True):
    key = (bpc, h, block, out_bf16)
    if key not in _NC_CACHE:
        _NC_CACHE[key] = build_nc(bpc, h, block, out_bf16)
    return _NC_CACHE[key]


def split_parity(x_f32):
    """[b, c, h, w] f32 -> bf16 [b, 2*c, h//2, w+2]: row parity split plus
    zero border columns; partition k = 64*p + c (p=0 even rows, p=1 odd)."""
    import ml_dtypes

    b, c, h, w = x_f32.shape
    xb = x_f32.astype(ml_dtypes.bfloat16)
    xP = np.zeros((b, 2, c, h // 2, w + 2), dtype=ml_dtypes.bfloat16)
    xP[:, 0, :, :, 1:-1] = xb[:, :, 0::2]
    xP[:, 1, :, :, 1:-1] = xb[:, :, 1::2]
    return xP.reshape(b, 2 * c, h // 2, w + 2)


def merge_parity(outP):
    """[b, 2, 128, hgs, 4, 2, w] (any float dtype) -> fp32 [b, 64, h, w]
    with partition k = 64*ph + c, parity p = ph ^ uh and image row
    2*(16*hg + 4*uu + 2*uh + up) + p = 32*hg + 8*uu + 4*uh + 2*up + p."""
    b, _, _, hgs, _, _, w = outP.shape
    o = np.asarray(outP, dtype=np.float32).reshape(b, 2, 2, 64, hgs, 4, 2, w)
    full = np.empty((b, 64, hgs, 4, 2, 2, 2, w), dtype=np.float32)
    for uh in range(2):
        for ph in range(2):
            full[:, :, :, :, uh, :, ph ^ uh, :] = o[:, uh, ph]
    return full.reshape(b, 64, 32 * hgs, w)


def kernel(x, weight):
    import ml_dtypes
    from concourse import bass_utils

    x = np.asarray(x, dtype=np.float32)
    weight = np.asarray(weight, dtype=np.float32)
    assert x.shape == (B_FULL, IN_F, H_FULL, W), x.shape

    xP = split_parity(x)
    wpack = pack_weights(normalize_weight(weight)).astype(ml_dtypes.bfloat16)
    bpc = B_FULL // N_CORES
    nc = _get_nc(bpc, H_FULL)
    in_maps = [
        {"x": xP[i * bpc : (i + 1) * bpc], "wpack": wpack} for i in range(N_CORES)
    ]
    res = bass_utils.run_bass_kernel_spmd(nc, in_maps, core_ids=list(range(N_CORES)))
    return np.concatenate([merge_parity(r["out"]) for r in res.results], axis=0)
